# revision 14
# baseline (speedup 1.0000x reference)
"""DenseCorr2d full kernel for 8 Trainium2 NeuronCores.

Reference computation (per example b):
  corr[(cm*16+ct), y, x] = sum_{dy,dx} tm_edgepad[cm, y+dy, x+dx] * tp[ct, dy, dx]
  out[co, y, x] = bias[co] + sum_{ci,ky,kx} W[co, ci, ky, kx] * corr_zpad[ci, y+ky-1, x+kx-1]

Sharding: data-parallel over batch; core i computes example i entirely.

Stage A (dense correlation) folds the template taps into the matmul
contraction dim: with y = 8w + j and dx = 4a + de, the contraction rows are
(f = j+dy, de) = 92 partitions, the stationary columns are (ct, j) = 128
(fully dense), and accumulation over a happens in PSUM (4 matmuls per
128-col x 512-spatial tile).  The moving operand baseT[cm, (f,de), w, x'] =
tm_pad[cm, 8w+f, x'+de] is precomputed on host so each tile load is a
contiguous-per-partition DMA.

corr lives in SBUF j-interleaved: corr[ci, (c, j, ws, x)] holds row
y = 8*(ws-1) + j of chunk c (ws=0/17 are the y=-1/128 zero-pad rows, x is
130 wide with zero x-borders).  This makes the PSUM evacuation a full-rate
copy (cast fp32->bf16 into a staging tile) plus ONE contiguous-run DMA per
cm, and keeps every stage-B tap window a contiguous slice within a slab.

Stage B runs the 3x3 'same' merge conv over residue bands (fixed j0, a
window of w): 18 PSUM-accumulating matmuls (9 taps x 2 input-channel
chunks) per band, tap shifts being flat offsets into the slab; bias fused
into the ScalarE PSUM->SBUF copy, which scatters rows back into the
row-major output tile.

All matmuls run in bf16 (inputs are unit-normal; accumulation in fp32
PSUM keeps the relative error ~4e-3, well inside the 2e-2 gate).
"""

from contextlib import ExitStack

import ml_dtypes
import numpy as np

import concourse.bass as bass
import concourse.tile as tile
from concourse import bacc, mybir
from concourse.bass_utils import run_bass_kernel_spmd

F32 = mybir.dt.float32
BF16 = mybir.dt.bfloat16

N_CORES = 8
# Problem shapes (hardcoded per contract).
B, CT, HT, WT = 8, 16, 16, 16
CM, HM, WM = 16, 128, 128
COUT, K = 64, 3
HP = HM + HT - 1  # 143 padded image rows/cols
NF = 23  # f = j + dy range
NP = 4 * NF  # 92 contraction rows (f, de)
XW = 140  # x' range of baseT (x + 4a <= 127+12)
SLAB = 18 * 130  # corr slab (c, j): 18 ws-rows of 130
NCORR = 16 * SLAB  # 2 chunks * 8 j

# stage-B w-window split: 16 w values per j0-residue, psum <= 512 fp32
RWS = [3, 3, 3, 3, 2, 2]
W0S = [0, 3, 6, 9, 12, 14]

_CACHE: dict = {}


def _emit(ctx: ExitStack, tc, nc, btT, sa2, wst, bia, zzb, out):
    const = ctx.enter_context(tc.tile_pool(name="const", bufs=1))
    corrp = ctx.enter_context(tc.tile_pool(name="corrp", bufs=1))

    sa2_sb = const.tile([NP, 4, 128], BF16, name="sa2_sb")
    nc.sync.dma_start(out=sa2_sb[:], in_=sa2.ap())
    w_sb = const.tile([128, 18, COUT], BF16, name="w_sb")
    nc.scalar.dma_start(out=w_sb[:, :9], in_=wst.ap()[:, :9])
    nc.scalar.dma_start(out=w_sb[:, 9:], in_=wst.ap()[:, 9:])
    b_sb = const.tile([COUT, 1], F32, name="b_sb")
    nc.scalar.dma_start(out=b_sb[:], in_=bia.ap())

    corr_sb = corrp.tile([128, NCORR + 2], BF16, name="corr_sb")
    corr_flat = corr_sb[:]
    # slab view: [p, c*8+j, ws*130+x]
    corr_j = corr_sb[:, :NCORR].rearrange("p (s t) -> p s t", s=16)
    # tail slack (zero-pad rows/cols arrive with the per-cm slab shuffle)
    nc.scalar.dma_start(out=corr_sb[:, NCORR:], in_=zzb.ap()[:, :2])

    # ---- Stage A ----
    with (
        tc.tile_pool(name="btp", bufs=2) as btp,
        tc.tile_pool(name="stp0", bufs=2) as stp0,
        tc.tile_pool(name="stp1", bufs=2) as stp1,
        tc.tile_pool(name="psA", bufs=8, space="PSUM") as psA,
    ):
        stps = [stp0, stp1]

        def load_pair(pr):
            # one big dma_start per cm: the HW splits an InstDMACopy across
            # SDMA engines at packet granularity, so big DMAs spread wide.
            t = btp.tile([NP, 2, 16, XW], BF16, name="bt", tag="bt")
            nc.sync.dma_start(out=t[:, 0], in_=btT.ap()[pr, :, 0])
            nc.sync.dma_start(out=t[:, 1], in_=btT.ap()[pr, :, 1])
            return t

        bt = load_pair(0)
        for pr in range(8):
            # prefetch the next pair BEFORE this pair's shuffle joins the
            # sync queue (HWDGE executes FIFO per queue).
            bt_next = load_pair(pr + 1) if pr < 7 else None
            pts = [
                [
                    psA.tile([128, 4, 128], F32, name=f"pA{cm2}_{ybg}", tag="pA")
                    for ybg in range(4)
                ]
                for cm2 in range(2)
            ]
            for a in range(4):
                for cm2 in range(2):
                    for ybg in range(4):
                        nc.tensor.matmul(
                            pts[cm2][ybg][:],
                            sa2_sb[:, a, :],
                            bt[:, cm2, 4 * ybg : 4 * ybg + 4, 4 * a : 4 * a + 128],
                            start=(a == 0),
                            stop=(a == 3),
                        )
            for cm2 in range(2):
                cm = 2 * pr + cm2
                # st covers the full 18-row slab: ws=0/17 zero-pad rows and
                # x-border zeros ride along with the data in one shuffle.
                st = stps[cm2].tile([128, 18, 130], BF16, name="st", tag="st")
                nc.gpsimd.memset(st[:, 0, :], 0.0)
                nc.gpsimd.memset(st[:, 17, :], 0.0)
                nc.gpsimd.memset(st[:, 1:17, 0], 0.0)
                nc.gpsimd.memset(st[:, 1:17, 129], 0.0)
                for ybg in range(4):
                    dst_sl = st[:, 1 + 4 * ybg : 5 + 4 * ybg, 1:129]
                    if ybg % 2 == 0:
                        nc.vector.tensor_copy(dst_sl, pts[cm2][ybg][:])
                    else:
                        nc.scalar.copy(dst_sl, pts[cm2][ybg][:])
                # st[(8ct+j), ws, x] -> corr slabs (c, j) whole
                c, cmh = cm // 8, cm % 8
                eng = nc.sync if cm2 == 0 else nc.scalar
                eng.dma_start(
                    out=corr_j[16 * cmh : 16 * cmh + 16, c * 8 : c * 8 + 8, :],
                    in_=st[:],
                )
            bt = bt_next

    # ---- Stage B ----
    def slab_off(c, j0, ky, w0):
        jj = j0 + ky - 1
        if jj < 0:
            return (c * 8 + 7) * SLAB + w0 * 130
        if jj > 7:
            return c * 8 * SLAB + (w0 + 2) * 130
        return (c * 8 + jj) * SLAB + (w0 + 1) * 130

    with (
        tc.tile_pool(name="psB", bufs=4, space="PSUM") as psB,
        tc.tile_pool(name="outp", bufs=2) as outp,
    ):
        for wg in range(len(RWS)):
            w0, rw = W0S[wg], RWS[wg]
            ot = outp.tile([COUT, 8 * rw, WM], F32, name="ot", tag="ot")
            for j0 in range(8):
                n = rw * 130
                pb = psB.tile([COUT, 3 * 130], F32, name="pb", tag="pb")
                first = True
                for c in range(2):
                    for s in range(9):
                        ky, kx = divmod(s, 3)
                        off = slab_off(c, j0, ky, w0) + kx
                        nc.tensor.matmul(
                            pb[:, :n],
                            w_sb[:, c * 9 + s, :],
                            corr_flat[:, off : off + n],
                            start=first,
                            stop=(c == 1 and s == 8),
                        )
                        first = False
                nc.scalar.activation(
                    ot[:, j0 : j0 + 8 * (rw - 1) + 1 : 8, :],
                    pb[:, :n].rearrange("p (a b) -> p a b", b=130)[:, :, 0:128],
                    mybir.ActivationFunctionType.Identity,
                    bias=b_sb[:, 0:1],
                )
            nc.sync.dma_start(
                out=out.ap()[:, 8 * w0 : 8 * w0 + 8 * rw, :],
                in_=ot[:],
            )


def _build():
    nc = bacc.Bacc("TRN2", target_bir_lowering=False, debug=False)
    btT = nc.dram_tensor("btT", [8, NP, 2, 16, XW], BF16, kind="ExternalInput")
    sa2 = nc.dram_tensor("sa2", [NP, 4, 128], BF16, kind="ExternalInput")
    wst = nc.dram_tensor("wst", [128, 18, COUT], BF16, kind="ExternalInput")
    bia = nc.dram_tensor("bias", [COUT, 1], F32, kind="ExternalInput")
    zzb = nc.dram_tensor("zzb", [128, 130], BF16, kind="ExternalInput")
    out = nc.dram_tensor("out", [COUT, HM, WM], F32, kind="ExternalOutput")
    with tile.TileContext(nc) as tc, ExitStack() as ctx:
        _emit(ctx, tc, nc, btT, sa2, wst, bia, zzb, out)
    nc.compile()
    return nc


def _get_nc():
    if "nc" not in _CACHE:
        _CACHE["nc"] = _build()
    return _CACHE["nc"]


def _host_prep(template, tomatch, W, b):
    template = np.ascontiguousarray(template, dtype=np.float32)
    tomatch = np.ascontiguousarray(tomatch, dtype=np.float32)
    W = np.ascontiguousarray(W, dtype=np.float32)
    b = np.ascontiguousarray(b, dtype=np.float32)
    bf = ml_dtypes.bfloat16

    tm_pad = np.pad(
        tomatch, ((0, 0), (0, 0), (0, HT - 1), (0, WT - 1)), mode="edge"
    )  # [B, CM, 143, 143]

    # baseT[b, cm, 4f+de, w, x'] = tm_pad[b, cm, 8w+f, x'+de]
    s0, s1, s2, s3 = tm_pad.strides
    bview = np.lib.stride_tricks.as_strided(
        tm_pad,
        shape=(B, CM, NF, 4, 16, XW),
        strides=(s0, s1, s2, s3, 8 * s2, s3),
    )
    # pair-grouped for one-DMA-per-pair loads: [b, pr, p, cm2, w, x']
    btT = (
        np.ascontiguousarray(bview)
        .reshape(B, 8, 2, NP, 16, XW)
        .transpose(0, 1, 3, 2, 4, 5)
        .astype(bf)
    )
    btT = np.ascontiguousarray(btT)

    # sa2[b, 4f+de, a, 8ct+j] = template[b, ct, f-j, 4a+de] for 0<=f-j<16
    sa2 = np.zeros((B, NP, 4, 128), np.float32)
    tview = template.reshape(B, CT, HT, 4, 4)  # [b, ct, dy, a, de]
    for j in range(8):
        for dy in range(HT):
            f = j + dy
            # [b, de, a, ct] slab
            sa2[:, 4 * f : 4 * f + 4, :, j::8] = tview[:, :, dy].transpose(
                0, 3, 2, 1
            )
    sa2 = sa2.astype(bf)

    # wst[k, c*9 + ky*3 + kx, co] = W[co, c*128+k, ky, kx]
    wst = np.ascontiguousarray(
        W.reshape(COUT, 2, 128, K, K).transpose(2, 1, 3, 4, 0).reshape(128, 18, COUT)
    ).astype(bf)
    bias = np.ascontiguousarray(b.reshape(COUT, 1))
    zzb = np.zeros((128, 130), bf)
    return btT, sa2, wst, bias, zzb


def _in_maps(template, tomatch, W, b):
    btT, sa2, wst, bias, zzb = _host_prep(template, tomatch, W, b)
    return [
        {"btT": btT[i], "sa2": sa2[i], "wst": wst, "bias": bias, "zzb": zzb}
        for i in range(N_CORES)
    ]


def kernel(template, tomatch, W, b):
    in_maps = _in_maps(template, tomatch, W, b)
    nc = _get_nc()
    res = run_bass_kernel_spmd(nc, in_maps, list(range(N_CORES)))
    return np.stack([res.results[i]["out"] for i in range(N_CORES)])
